# revision 39
# baseline (speedup 1.0000x reference)
"""CRF log-loss kernel for TRN2, data-parallel over batch on 8 NeuronCores.

Forward algorithm with warmup chains:
  * S=512 steps split into 8 segments of L=64. One FORWARD chain per
    segment; chains 1-7 start W=2 steps early from the ones vector and
    converge to the true state direction before their segment begins
    (error ~ (lambda2/lambda1)^W, far below tolerance). All 8 chains run
    concurrently: R = L + W = 66 sequential rounds instead of 512.
  * Chains are stacked in pairs on the 128 SBUF partitions (2 chains of
    T=64 tags), giving 4 independent pair-blocks per round: 4 [128x128]
    matmuls against a resident block-diagonal transition stationary + 4
    DVE multiplies with the exp'd emissions. Four parallel streams keep
    both engines' pipelines deep enough to hide the PSUM access bubble
    and the cross-engine semaphore latency.
  * Emissions are laid out round-major across blocks, so each exp group
    is a single [128, 5632] fp8 DMA and two scalar-engine Exp calls with
    a host-calibrated per-group bias (shared across each partition
    half's four chains; the telescoped readout corrects it exactly).
  * logZ is assembled from 15 probe dots (per-chain postwarm + end
    magnitudes, final chain contracted against exp(trans[stop])) via a
    single Ln activation; seam telescoping cancels the warmup segments.
  * Gold score: host gathers emission + transition + start terms into a
    single [BC, S] bf16 array; the device reduces it in one DVE op.
"""
import numpy as np
import ml_dtypes
from contextlib import ExitStack

import concourse.bass as bass
import concourse.bacc as bacc
import concourse.tile as tile
import concourse.mybir as mybir
from concourse.bass_utils import run_bass_kernel_spmd

bf16 = ml_dtypes.bfloat16
fp8 = ml_dtypes.float8_e4m3
f32 = mybir.dt.float32
bf16d = mybir.dt.bfloat16
f8 = mybir.dt.float8e4

B, S, T = 1024, 512, 64
NC = 8
BC = B // NC            # 128 examples per core
NSEG = 8                # segments / chains
L = S // NSEG           # 64 steps per segment
W = 2                   # warmup rounds for chains 1..7
R = L + W               # 66 rounds
NB = NSEG // 2          # 4 pair-blocks
RBC = NB * BC           # 512 columns per round
GRP = 11                # rounds per exp group
NGRP = R // GRP         # 6 groups
GBC = GRP * RBC         # 5632 columns per group
SPL = 6 * RBC           # split point inside a group (6 rounds)
PAD = -240.0            # fp8 pad value: exp() underflows to 0 in bf16

AF = mybir.ActivationFunctionType
ALU = mybir.AluOpType


def _build_program():
    nc = bacc.Bacc("TRN2", target_bir_lowering=False, debug=False, num_devices=NC)

    ftp_d = nc.dram_tensor("ftp", [128, (R - GRP) * RBC], f8,
                           kind="ExternalInput")
    st0_d = nc.dram_tensor("st0", [128, RBC], bf16d, kind="ExternalInput")
    et0_d = nc.dram_tensor("et0", [128, GBC], bf16d, kind="ExternalInput")
    constb_d = nc.dram_tensor("constb", [128, 132 + RBC], bf16d,
                              kind="ExternalInput")
    biasf_d = nc.dram_tensor("biasf", [128, NGRP + 1], f32, kind="ExternalInput")
    egc_d = nc.dram_tensor("egc", [BC, S], bf16d, kind="ExternalInput")
    out_d = nc.dram_tensor("out", [BC, 1], f32, kind="ExternalOutput")

    with tile.TileContext(nc) as tc, ExitStack() as ctx:
        sbpool = ctx.enter_context(tc.tile_pool(name="sb", bufs=2))
        pspool = ctx.enter_context(tc.tile_pool(name="ps", bufs=NB, space="PSUM"))
        psdpool = ctx.enter_context(tc.tile_pool(name="psd", bufs=1, space="PSUM"))
        cpool = sbpool   # const-ish rings live in sbpool (one buf used)

        # ---- startup: group-0 emissions come pre-exp'd (bf16) from the
        # host. DMA in need-ordered chunks — all 16 queues serve every
        # dma_start, so issue order IS priority order ----
        st0_s = sbpool.tile([128, RBC], bf16d, name="st0")
        nc.sync.dma_start(st0_s[:, :], st0_d[:, :])
        constb_s = cpool.tile([128, 132 + RBC], bf16d, name="cb")
        nc.sync.dma_start(constb_s[:, :], constb_d[:, :])
        ett0 = sbpool.tile([128, GBC], bf16d, name="et")
        nc.sync.dma_start(ett0[:, RBC:4 * RBC], et0_d[:, RBC:4 * RBC])
        biasf_s = cpool.tile([128, NGRP + 1], f32, name="bf")
        nc.sync.dma_start(biasf_s[:, :], biasf_d[:, :])
        nc.sync.dma_start(ett0[:, 4 * RBC:8 * RBC], et0_d[:, 4 * RBC:8 * RBC])
        nc.sync.dma_start(ett0[:, 8 * RBC:], et0_d[:, 8 * RBC:GBC])

        bd_ap = constb_s[:, 0:128]
        probes = constb_s[:, 128:132]   # [ones_top, ones_bot, ones_top, p_bot]
        inits = [constb_s[:, 132 + i * BC:132 + (i + 1) * BC] for i in range(NB)]
        ett = ett0

        egc_s = cpool.tile([BC, S], bf16d, name="egc")
        egsum = cpool.tile([BC, 1], f32, name="egs")
        egsum2 = cpool.tile([BC, 1], f32, name="egs2")
        egsum3 = cpool.tile([BC, 1], f32, name="egs3")
        psd = psdpool.tile([128, 512], f32, name="psd")
        lns = cpool.tile([128, 15], f32, name="lns")
        r2 = cpool.tile([128, 1], f32, name="r2")

        # ---- main rounds ----
        stprev = [None] * NB

        def stref(i):
            return stprev[i][:, :]

        next_ett = None
        for r in range(R):
            g = r // GRP
            sl = (r % GRP) * RBC
            if r % GRP == 0 and r > 0:
                ett = next_ett
            if r % GRP == 2 and g + 1 < NGRP:
                gn = g + 1
                ft = sbpool.tile([128, GBC], f8, name="ft")
                base = (gn - 1) * GBC
                nc.sync.dma_start(ft[:, 0:SPL], ftp_d[:, base:base + SPL])
                nc.sync.dma_start(ft[:, SPL:], ftp_d[:, base + SPL:base + GBC])
                next_ett = sbpool.tile([128, GBC], bf16d, name="et")
                bap = biasf_s[:, gn:gn + 1]
                nc.scalar.activation(next_ett[:, 0:SPL], ft[:, 0:SPL], AF.Exp,
                                     bias=bap)
                nc.scalar.activation(next_ett[:, SPL:], ft[:, SPL:], AF.Exp,
                                     bias=bap)
            if r == 4:
                nc.sync.dma_start(egc_s[:, :], egc_d[:, :])
            if r == 0:
                continue   # round-0 state (init * e_0) ships from the host
            for i in range(NB):
                st = sbpool.tile([128, BC], bf16d, name=f"st{i}")
                ps = pspool.tile([128, 512], f32, name="ps")
                if r == 1:
                    rhs = st0_s[:, i * BC:(i + 1) * BC]
                else:
                    rhs = stprev[i][:, :]
                nc.tensor.matmul(ps[:, 0:BC], bd_ap, rhs, start=True, stop=True)
                nc.vector.tensor_tensor(
                    st[:, :], ps[:, 0:BC],
                    ett[:, sl + i * BC:sl + (i + 1) * BC], ALU.mult)
                stprev[i] = st
            if r == W - 1:
                # postwarm magnitudes: chains 1..7 (psd cols 0..6)
                nc.tensor.matmul(psd[:, 0:1], stref(0), probes[:, 1:2],
                                 start=True, stop=True)
                for i in range(1, NB):
                    nc.tensor.matmul(psd[:, 2 * i - 1:2 * i + 1], stref(i),
                                     probes[:, 0:2], start=True, stop=True)
            if r == L - 1:
                # chain 0 ends (psd col 7); pad rounds follow. The
                # postwarm lns + reduce run here on idle engine slack.
                nc.tensor.matmul(psd[:, 7:8], stref(0), probes[:, 0:1],
                                 start=True, stop=True)
                nc.scalar.activation(lns[:, 0:8], psd[:, 0:8], AF.Ln)
                nc.vector.tensor_reduce(r2[:, :], lns[:, 0:7],
                                        axis=mybir.AxisListType.X, op=ALU.add)

        # ---- finals: chains 1..7 end dots (psd cols 8..14); the egc
        # reduce rides the vector engine while the dots run ----
        nc.tensor.matmul(psd[:, 8:9], stref(0), probes[:, 1:2],
                         start=True, stop=True)
        for i in range(1, NB - 1):
            nc.tensor.matmul(psd[:, 7 + 2 * i:9 + 2 * i], stref(i),
                             probes[:, 0:2], start=True, stop=True)
        nc.tensor.matmul(psd[:, 13:15], stref(NB - 1), probes[:, 2:4],
                         start=True, stop=True)
        r1 = cpool.tile([128, 1], f32, name="r1")
        lout = cpool.tile([BC, 1], f32, name="lout")
        nc.vector.tensor_reduce(egsum[:, :], egc_s[:, :],
                                axis=mybir.AxisListType.X, op=ALU.add)
        nc.vector.tensor_sub(egsum2[:, :], egsum[:, :],
                             biasf_s[:, NGRP:NGRP + 1])
        nc.vector.tensor_add(egsum3[:, :], egsum2[:, :], r2[:, :])
        nc.scalar.activation(lns[:, 8:15], psd[:, 8:15], AF.Ln)
        nc.vector.tensor_reduce(r1[:, :], lns[:, 7:15],
                                axis=mybir.AxisListType.X, op=ALU.add)
        nc.vector.tensor_sub(lout[:, :], r1[:, :], egsum3[:, :])
        nc.sync.dma_start(out_d[:, :], lout[:, :])

    nc.compile()
    return nc


def _chain_schedule():
    """step_of[q, r], valid[q, r] for the NSEG chains over R rounds."""
    step_of = np.zeros((NSEG, R), dtype=np.int64)
    valid = np.ones((NSEG, R), dtype=bool)
    step_of[0, :L] = np.arange(L)
    valid[0, L:] = False
    for q in range(1, NSEG):
        step_of[q, :W] = np.arange(q * L - W, q * L)
        step_of[q, W:] = np.arange(q * L, (q + 1) * L)
    return step_of, valid


def _calibrate_beta(feats, transitions, start_tag, n_cal=8):
    """Per-step mean log-growth of the forward recursion from a few
    examples, used as compile-free device bias constants."""
    Tm = np.exp(transitions.astype(np.float64))
    idx = np.linspace(0, B - 1, n_cal).astype(np.int64)
    u = np.tile(np.exp(start_tag.astype(np.float64))[None, :], (n_cal, 1))
    growth = np.zeros((n_cal, S))
    f = feats[idx].astype(np.float64)
    for s in range(S):
        u2 = np.exp(f[:, s, :]) * (u @ Tm.T)
        z = u2.sum(axis=1)
        growth[:, s] = np.log(z)
        u = u2 / z[:, None]
    return growth.mean(axis=0)  # [S]


def _host_prep(feats, transitions, start_tag, tags):
    """Shared (cross-core) constants + per-core tensors."""
    Tm = np.exp(transitions.astype(np.float64))
    beta_step = _calibrate_beta(feats, transitions, start_tag)
    step_of, valid = _chain_schedule()

    # exp bias per (chain, group); shared across each partition half's
    # chains (bias is per-partition, uniform over columns), f32 (exactly
    # what the device applies)
    bias_qg = np.zeros((NSEG, NGRP), dtype=np.float32)
    for q in range(NSEG):
        for g in range(NGRP):
            rr = np.arange(g * GRP, (g + 1) * GRP)
            ok = valid[q, rr]
            if ok.any():
                bias_qg[q, g] = -beta_step[step_of[q, rr[ok]]].mean()
    for par in range(2):
        qs = [q for q in range(NSEG) if q % 2 == par]
        m = bias_qg[qs].mean(axis=0)
        for q in qs:
            bias_qg[q] = m
    bias_round = np.repeat(bias_qg.astype(np.float64), GRP, axis=1)  # [NSEG, R]
    Bq = np.cumsum(np.where(valid, bias_round, 0.0), axis=1)
    C = -(sum(Bq[q, R - 1] for q in range(1, NSEG)) + Bq[0, L - 1]
          - sum(Bq[q, W - 1] for q in range(1, NSEG)))

    # constb: bd(128) | probes(4) | init(NB*BC)
    bd = np.zeros((128, 128), dtype=np.float64)
    bd[:T, :T] = Tm.T
    bd[T:, T:] = Tm.T
    probes = np.zeros((128, 4), dtype=np.float64)
    probes[:T, 0] = 1.0
    probes[T:, 1] = 1.0
    probes[:T, 2] = 1.0
    probes[T:, 3] = Tm[T - 1, :]
    u0 = np.exp(start_tag.astype(np.float64))
    tm1 = Tm.sum(axis=1)
    init = np.zeros((128, NB * BC), dtype=np.float64)
    for i in range(NB):
        top = Tm @ u0 if i == 0 else tm1
        init[:T, i * BC:(i + 1) * BC] = top[:, None]
        init[T:, i * BC:(i + 1) * BC] = tm1[:, None]
    constb = np.concatenate([bd, probes, init], axis=1).astype(bf16)

    biasf = np.zeros((128, NGRP + 1), dtype=np.float32)
    for g in range(NGRP):
        biasf[:T, g] = bias_qg[0, g]
        biasf[T:, g] = bias_qg[1, g]
    biasf[:, NGRP] = np.float32(C)

    # transposed emissions, round-major: ftp[p, r*RBC + i*BC + b].
    # Group 0 ships pre-exp'd bf16 (et0); groups 1+ ship raw fp8.
    fs = np.ascontiguousarray(feats.transpose(1, 2, 0))  # [S, T, B]
    ftp_full = np.full((128, R, NB, B), PAD, dtype=np.float32)
    for q in range(NSEG):
        i, bot = divmod(q, 2)
        rows = slice(0, T) if bot == 0 else slice(T, 128)
        nr = L if q == 0 else R
        ftp_full[rows, :nr, i, :] = fs[step_of[q, :nr]].transpose(1, 0, 2)
    bias0 = np.concatenate([np.full(T, bias_qg[0, 0], np.float64),
                            np.full(T, bias_qg[1, 0], np.float64)])
    et0_raw = np.exp(ftp_full[:, :GRP].astype(np.float64)
                     + bias0[:, None, None, None])
    et0_full = et0_raw.astype(bf16)
    iv = np.zeros((128, NB), dtype=np.float64)
    for i in range(NB):
        iv[:T, i] = Tm @ u0 if i == 0 else tm1
        iv[T:, i] = tm1
    st0_full = (et0_raw[:, 0] * iv[:, :, None]).astype(bf16)  # [128, NB, B]
    ftp_rest = ftp_full[:, GRP:].astype(fp8)

    # gold score, host-gathered: emission + transition + start terms
    tg = tags.astype(np.int64)
    egc = np.take_along_axis(feats.astype(np.float32), tg[:, :, None],
                             axis=2)[:, :, 0]                       # [B, S]
    egc[:, 1:] += transitions[tg[:, :-1], tg[:, 1:]]
    egc[:, 0] += start_tag[tg[:, 0]] + start_tag[tg[:, -1]]
    egc = egc.astype(bf16)

    shared = dict(constb=constb, biasf=biasf)
    in_maps = []
    for c in range(NC):
        sl = slice(c * BC, (c + 1) * BC)
        ftp = np.ascontiguousarray(
            ftp_rest[:, :, :, sl]).reshape(128, (R - GRP) * RBC)
        et0 = np.ascontiguousarray(et0_full[:, :, :, sl]).reshape(128, GBC)
        st0 = np.ascontiguousarray(st0_full[:, :, sl]).reshape(128, RBC)
        im = {"ftp": ftp, "et0": et0, "st0": st0,
              "egc": np.ascontiguousarray(egc[sl])}
        im.update(shared)
        in_maps.append(im)
    return in_maps


_NC_CACHE = {}


def _get_program():
    if "nc" not in _NC_CACHE:
        _NC_CACHE["nc"] = _build_program()
    return _NC_CACHE["nc"]


def kernel(feats, transitions, start_tag, tags, mask_x, len_seq):
    feats = np.asarray(feats, dtype=np.float32)
    transitions = np.asarray(transitions, dtype=np.float32)
    start_tag = np.asarray(start_tag, dtype=np.float32)
    tags_np = np.asarray(tags)

    in_maps = _host_prep(feats, transitions, start_tag, tags_np)
    nc = _get_program()
    res = run_bass_kernel_spmd(nc, in_maps, list(range(NC)))
    out = np.concatenate([res.results[i]["out"][:, 0] for i in range(NC)])
    return out.astype(np.float32)
